# revision 19
# baseline (speedup 1.0000x reference)
"""Multi-head causal attention (B=2, S=2048, D=1024, H=16) on 8 trn2 NeuronCores.

Strategy (tensor-parallel over heads, per the sharding hint):
  - Each core owns 2 heads (128 of 1024 hidden dims): W_q/W_k/W_v column-parallel.
  - Activations kept transposed ([dim, token]) end to end so every matmul
    contracts on the partition axis with zero on-device transposes of x.
  - Fully software-pipelined: for each 512-token tile, project Q/K/V,
    transpose V, then run that q-tile's causal attention - the PE never waits
    for a separate projection phase.
  - scores^T = K^T.T @ Q^T per 128-key-chunk x 512-query-tile, two heads packed
    into disjoint PE row-groups (contraction is only dk=64).
  - softmax without max-subtraction (scores are O(1)); rowsum folded into the
    PV matmul via an augmented V [keys, 64+1] whose last column is ones.
  - exp only on the causal part of diagonal chunks; the rest of the P tile is
    zeroed, and only the 128-wide diagonal strip is tri-masked.
  - reciprocals batched into one tiny [128, 8] DVE op per q-tile; row broadcast
    on the otherwise-idle GpSimd engine.
  - ctx re-sharded token-parallel with one AllToAll per batch element; the
    batch-0 AllToAll overlaps batch-1 attention. Out-projection runs with full
    W_o on each core for its 2x256 tokens.
  - bf16 matmul inputs everywhere; PSUM accumulation and softmax
    normalization stay fp32.

kernel(**inputs) takes the full unsharded inputs and returns the full output.
"""

import numpy as np
import ml_dtypes

import concourse.bass as bass
import concourse.mybir as mybir
import concourse.tile as tile
from concourse import bacc
from concourse.bass import ts
from concourse.bass_utils import run_bass_kernel_spmd
from concourse.tile_rust import add_dep_helper

B, S, D = 2, 2048, 1024
H, DK = 16, 64
NCORE = 8
T = B * S          # 4096 tokens
TT = 512           # token tile (projections, q-tiles)
NT = T // TT       # 8
KC = 128           # key chunk
NJ = S // TT       # 4 q-tiles per batch
GG = 256           # a2a token group (per dst core, per batch)
SCALE = 1.0 / np.sqrt(DK)

f32 = mybir.dt.float32
bf16 = mybir.dt.bfloat16
EXP = mybir.ActivationFunctionType.Exp
MULT = mybir.AluOpType.mult
npbf = ml_dtypes.bfloat16


def build_program():
    nc = bacc.Bacc("TRN2", target_bir_lowering=False, debug=False,
                   num_devices=NCORE)

    xT_d = nc.dram_tensor("xT", [NT, 128, 8, TT], bf16, kind="ExternalInput").ap()
    wT_d = nc.dram_tensor("wT", [128, 8, 3, 128], bf16, kind="ExternalInput").ap()
    woT_d = nc.dram_tensor("woT", [8, 128, 8, 128], bf16, kind="ExternalInput").ap()
    bqkv_d = nc.dram_tensor("bqkv", [128, 3], f32, kind="ExternalInput").ap()
    bo_d = nc.dram_tensor("bo", [128, 8], f32, kind="ExternalInput").ap()
    trimask_d = nc.dram_tensor("trimask", [128, 128], bf16, kind="ExternalInput").ap()
    ident_d = nc.dram_tensor("ident", [128, 128], bf16, kind="ExternalInput").ap()
    # out^T for this core's tokens: [od_tile, p, batch, 256]
    outT_d = nc.dram_tensor("outT", [8, 128, B, 2, KC], f32, kind="ExternalOutput").ap()

    with tile.TileContext(nc) as tc:
        with (
            tc.tile_pool(name="const", bufs=1) as constp,
            tc.tile_pool(name="wostream", bufs=8) as wop,
            tc.tile_pool(name="xstream", bufs=2) as xp,
            tc.tile_pool(name="qkv", bufs=NT) as qkvp,
            tc.tile_pool(name="vaug", bufs=NJ) as vaugp,
            tc.tile_pool(name="ptile", bufs=4) as pp,
            tc.tile_pool(name="post", bufs=2) as postp,
            tc.tile_pool(name="cxn", bufs=4) as cxnp,
            tc.tile_pool(name="cxhold", bufs=4) as cxp,
            tc.tile_pool(name="outsb", bufs=2) as outp,
            tc.tile_pool(name="ps_s", bufs=2, space="PSUM") as ps_s,
            tc.tile_pool(name="ps_ctx", bufs=1, space="PSUM") as ps_ctx,
            tc.tile_pool(name="ps_misc", bufs=2, space="PSUM") as ps_misc,
            tc.tile_pool(name="dram", bufs=1, space="DRAM") as dramp,
        ):
            # ---- constants ----
            wT = constp.tile([128, 8, 3, 128], bf16, tag="wT")
            nc.sync.dma_start(wT[:], wT_d)
            bqkv = constp.tile([128, 3], f32, tag="bqkv")
            nc.sync.dma_start(bqkv[:], bqkv_d)
            bo_sb = constp.tile([128, 8], f32, tag="bo")
            nc.sync.dma_start(bo_sb[:], bo_d)
            trimask = constp.tile([128, 128], bf16, tag="trimask")
            nc.sync.dma_start(trimask[:], trimask_d)
            ident = constp.tile([128, 128], bf16, tag="ident")
            nc.sync.dma_start(ident[:], ident_d)

            # per-token-tile Q/K/V (transposed) and per-tile augmented V
            qkv_t = [[None] * NT for _ in range(3)]   # [j][t] -> [128, TT]
            vaug_t = [[[None] * NJ for _ in range(2)] for _ in range(B)]

            # one A2A per batch: dst core c <- tokens [256c, +256) of each batch
            a2a_in = {b: dramp.tile([NCORE, 128, GG], bf16, name=f"a2a_in{b}")
                      for b in range(B)}
            a2a_out = {b: dramp.tile([NCORE, 128, GG], bf16, name=f"a2a_out{b}")
                       for b in range(B)}

            last_chain_dma = [None]

            def proj_tile(t):
                xt = xp.tile([128, 8, TT], bf16, tag="xt")
                if t == 0:
                    # split the first tile so the opening matmul starts after
                    # ~128KB instead of the full megabyte
                    for o in range(8):
                        nc.sync.dma_start(xt[:, o, :], xT_d[t, :, o, :])
                else:
                    nc.sync.dma_start(xt[:], xT_d[t])
                for j in range(3):
                    ps = ps_misc.tile([128, TT], f32, tag="mm")
                    for o in range(8):
                        nc.tensor.matmul(ps[:], wT[:, o, j, :], xt[:, o, :],
                                         start=(o == 0), stop=(o == 7))
                    qt = qkvp.tile([128, TT], bf16, tag=f"qkv{j}",
                                   name=f"qkv{j}_{t}")
                    nc.vector.tensor_scalar_add(qt[:], ps[:], bqkv[:, j:j + 1])
                    qkv_t[j][t] = qt

            def vtrans_tile(t):
                b, tl = t // NJ, t % NJ
                va = [vaugp.tile([128, NJ, DK + 1], bf16, tag=f"va{b}{h}",
                                 name=f"va{b}{h}_{tl}") for h in range(2)]
                for h in range(2):
                    nc.vector.memset(va[h][:, :, DK:DK + 1], 1.0)
                    vaug_t[b][h][tl] = va[h]
                for kt in range(NJ):
                    ps_t = ps_misc.tile([128, TT], bf16, tag="mm")
                    nc.tensor.transpose(ps_t[:, 0:128],
                                        qkv_t[2][t][:, kt * KC:(kt + 1) * KC],
                                        ident[:])
                    for h in range(2):
                        nc.vector.tensor_copy(va[h][:, kt, 0:DK],
                                              ps_t[:, DK * h:DK * h + DK])

            def attention_qtile(b, j):
                nk = 4 * (j + 1)
                pc = [ps_ctx.tile([DK + 1, TT], f32, tag=f"c{h}", name=f"pc{h}")
                      for h in range(2)]

                def emit_pv(p_tile, m):
                    for h in range(2):
                        nc.tensor.matmul(
                            pc[h][:], vaug_t[b][h][m // 4][:, m % 4, :],
                            p_tile[:, TT * h:TT * (h + 1)],
                            start=(m == 0), stop=(m == nk - 1),
                            skip_group_check=True)

                qt = qkv_t[0][b * NJ + j]
                pending = []
                for m in range(nk):
                    kt_tile = qkv_t[1][b * NJ + m // 4]
                    ko = (m % 4) * KC
                    ps = ps_s.tile([128, 2 * TT], f32, tag="s")
                    nc.tensor.matmul(ps[:, 0:TT], kt_tile[0:DK, ko:ko + KC],
                                     qt[0:DK, :],
                                     start=True, stop=True, tile_position=(0, 0))
                    nc.tensor.matmul(ps[:, TT:], kt_tile[DK:128, ko:ko + KC],
                                     qt[DK:128, :],
                                     start=True, stop=True, tile_position=(64, 0))
                    p = pp.tile([128, 2 * TT], bf16, tag="p")
                    r = m - 4 * j
                    if r >= 0:
                        if r > 0:
                            nc.vector.memset(
                                p[:].rearrange("k (h q) -> k h q", h=2)[:, :, 0:KC * r],
                                0.0)
                        nc.scalar.activation(
                            p[:].rearrange("k (h q) -> k h q", h=2)[:, :, KC * r:],
                            ps[:].rearrange("k (h q) -> k h q", h=2)[:, :, KC * r:],
                            EXP, scale=float(SCALE))
                        nc.vector.tensor_tensor(
                            p[:].rearrange("k (h q) -> k h q", h=2)[:, :, KC * r:KC * (r + 1)],
                            p[:].rearrange("k (h q) -> k h q", h=2)[:, :, KC * r:KC * (r + 1)],
                            trimask[:, None, :].to_broadcast([128, 2, 128]), MULT)
                    else:
                        nc.scalar.activation(p[:], ps[:], EXP, scale=float(SCALE))
                    pending.append((p, m))
                    if len(pending) > 2:   # depth-2: PE never waits on a fresh exp
                        emit_pv(*pending.pop(0))
                for pm in pending:
                    emit_pv(*pm)

                # per-q-tile softmax normalization + ship to the A2A buffer.
                # cx/rtmp copies come first so the ctx PSUM banks free up
                # before the DVE queue hits the DMA-gated reciprocal; the tiny
                # gather DMAs ride the idle GpSimd SWDGE channel instead of
                # queueing behind megabyte x-tile loads on Sync.
                rs_g = postp.tile([128, 8], f32, tag="rsg")
                cxs = []
                for h in range(2):
                    rtmp = cxnp.tile([1, TT], f32, tag="rtmp")
                    nc.vector.tensor_copy(rtmp[:], pc[h][DK:DK + 1, :])
                    cx = cxp.tile([DK, TT], f32, tag="cx")
                    nc.vector.tensor_copy(cx[:], pc[h][0:DK, :])
                    cxs.append(cx)
                    nc.gpsimd.dma_start(rs_g[:, 4 * h:4 * h + 4], rtmp[:])
                rc_g = postp.tile([128, 8], f32, tag="rcg")
                with nc.allow_low_precision(reason="softmax denominator"):
                    nc.vector.reciprocal(rc_g[:], rs_g[:])
                for h in range(2):
                    cx = cxs[h]
                    rrow = cxnp.tile([1, TT], f32, tag="rrow")
                    nc.gpsimd.dma_start(rrow[:], rc_g[:, 4 * h:4 * h + 4])
                    bcast = cxnp.tile([DK, TT], f32, tag="bcast")
                    nc.gpsimd.partition_broadcast(bcast[:], rrow[:], channels=DK)
                    cxn = cxnp.tile([DK, TT], bf16, tag="cxn")
                    nc.vector.tensor_tensor(cxn[:], cx[:], bcast[:], MULT)
                    for g in range(2):   # 256-token groups -> dst cores 2j+g
                        dma = nc.sync.dma_start(
                            a2a_in[b][2 * j + g, DK * h:DK * (h + 1), :],
                            cxn[:, GG * g:GG * (g + 1)])
                        last_chain_dma[0] = dma

            def do_a2a(b):
                nc.gpsimd.collective_compute(
                    "AllToAll", mybir.AluOpType.bypass,
                    replica_groups=[list(range(NCORE))],
                    ins=[a2a_in[b][:].opt()], outs=[a2a_out[b][:].opt()])

            ctx_tiles = {}
            chain_anchor = [None]

            def outproj(b, wo_tiles, orange, anchor):
                W = GG
                if b not in ctx_tiles:
                    ctx_sb = constp.tile([128, 8, W], bf16, tag=f"ctx{b}",
                                         name=f"ctx{b}")
                    # one DMA per source rank; gate behind the given chain
                    # anchor so the scheduler can't hoist the collective wait
                    # ahead of attention-critical DMAs on the same queue.
                    for d in range(8):
                        dma = nc.sync.dma_start(ctx_sb[:, d, :], a2a_out[b][d])
                        if anchor is not None:
                            add_dep_helper(dma.ins, anchor.ins, sync=False,
                                           reason="don't hoist a2a-gated ctx DMA")
                    ctx_tiles[b] = ctx_sb
                ctx_sb = ctx_tiles[b]
                for o in orange:
                    ps = ps_misc.tile([128, TT], f32, tag="mm")
                    for d in range(8):
                        nc.tensor.matmul(ps[:, 0:W], wo_tiles[o][:, d, :],
                                         ctx_sb[:, d, :],
                                         start=(d == 0), stop=(d == 7))
                    ot = outp.tile([128, W], f32, tag="ot")
                    nc.vector.tensor_scalar_add(ot[:], ps[:, 0:W], bo_sb[:, o:o + 1])
                    nc.sync.dma_start(
                        outT_d[o, :, b, :, :].rearrange("p a c -> p (a c)"),
                        ot[:])

            # ---- fully pipelined schedule (projection one tile ahead) ----
            proj_tile(0)
            wo_tiles = []
            for o in range(8):
                wo_t = wop.tile([128, 8, 128], bf16, tag="wo", name=f"wo{o}")
                nc.gpsimd.dma_start(wo_t[:], woT_d[o])
                wo_tiles.append(wo_t)
            for g in range(NT):
                if g + 1 < NT:
                    proj_tile(g + 1)
                vtrans_tile(g)
                attention_qtile(g // NJ, g % NJ)
                if g == NJ - 1:
                    do_a2a(0)
                    chain_anchor[0] = last_chain_dma[0]
            do_a2a(1)
            outproj(0, wo_tiles, range(8), last_chain_dma[0])
            outproj(1, wo_tiles, range(8), last_chain_dma[0])

    nc.compile()
    return nc


def make_in_maps(x, Wq, bq, Wk, bk, Wv, bv, Wo, bo):
    x = np.asarray(x, np.float32)
    xT = np.ascontiguousarray(x.reshape(T, D).T)                  # [D, T]
    # [NT, 128, 8, TT]: xT_t[t, p, o, q] = xT[o*128+p, t*TT+q]
    xT_t = np.ascontiguousarray(
        xT.reshape(8, 128, NT, TT).transpose(2, 1, 0, 3)).astype(npbf)

    woT = np.ascontiguousarray(
        np.asarray(Wo, np.float32).T.reshape(8, 128, 8, 128)
        .transpose(2, 1, 0, 3)).astype(npbf)

    trimask = (np.arange(128)[:, None] <= np.arange(128)[None, :]).astype(npbf)
    ident = np.eye(128, dtype=npbf)
    bo_t = np.ascontiguousarray(np.asarray(bo, np.float32).reshape(8, 128).T)

    in_maps = []
    for c in range(NCORE):
        sl = slice(128 * c, 128 * (c + 1))
        wT_c = np.stack(
            [np.ascontiguousarray(
                np.asarray(W, np.float32)[sl, :].T.reshape(8, 128, 128)
                .transpose(1, 0, 2))
             for W in (Wq, Wk, Wv)], axis=2)                       # [128, 8, 3, 128]
        bqkv_c = np.stack([np.asarray(b_, np.float32)[sl]
                           for b_ in (bq, bk, bv)], axis=1)        # [128, 3]
        in_maps.append({
            "xT": xT_t,
            "wT": np.ascontiguousarray(wT_c).astype(npbf),
            "woT": woT,
            "bqkv": np.ascontiguousarray(bqkv_c),
            "bo": bo_t,
            "trimask": trimask,
            "ident": ident,
        })
    return in_maps


def assemble_output(results):
    # results[c]["outT"]: [8, 128, B, 2*128] = out^T[od, (b, 256c..256c+256)]
    outT = np.empty((D, B, S), np.float32)
    for c in range(NCORE):
        outT[:, :, GG * c:GG * (c + 1)] = results[c]["outT"].reshape(D, B, GG)
    return np.ascontiguousarray(outT.transpose(1, 2, 0))


_PROGRAM = None


def get_program():
    global _PROGRAM
    if _PROGRAM is None:
        _PROGRAM = build_program()
    return _PROGRAM


def run(in_maps, **kwargs):
    nc = get_program()
    return run_bass_kernel_spmd(nc, in_maps, core_ids=list(range(NCORE)), **kwargs)


def kernel(x, Wq, bq, Wk, bk, Wv, bv, Wo, bo):
    in_maps = make_in_maps(x, Wq, bq, Wk, bk, Wv, bv, Wo, bo)
    res = run(in_maps)
    return assemble_output(res.results)


if __name__ == "__main__":
    rng = np.random.default_rng(0)
    x = rng.standard_normal((B, S, D), dtype=np.float32)
    mk = lambda *s: ((rng.random(s).astype(np.float32)) - 0.5) / 16
    out = kernel(x, mk(D, D), mk(D), mk(D, D), mk(D), mk(D, D), mk(D),
                 mk(D, D), mk(D))
    print(out.shape, out.dtype, np.abs(out).mean())
